# revision 38
# baseline (speedup 1.0000x reference)
"""Multi-head causal attention (B=2, S=2048, D=4096, H=32, hd=128) on 8 trn2 cores.

Sharding: DP over batch (2) x TP over heads (4 groups of 8 heads).
Core c: batch b = c//4, head-group tp = c%4.
Each core computes a partial output [2048, 4096] (wo row-sharded); host sums
the 4 partials per batch.

Data path is bf16 (inputs rounded host-side; all matmuls accumulate in fp32
PSUM), which halves DMA traffic and lets the full x [4096, 2048] strip stay
resident in SBUF so phase 1 makes a single pass over x with weights streamed
once. Softmax/normalization arithmetic stays fp32.
Host pre-transposes x / weights so every DMA is natural-layout.
q/k head dims are de-interleaved (evens then odds) on the host so RoPE becomes
full-tile DVE ops on partition halves; the permutation is consistent between
q and k so scores are unchanged. v / wo stay in natural order.
The causal mask enters as 4 distinct [128, 512] diagonal-block patterns kept
resident in SBUF (fully-masked blocks are skipped, fully-open blocks need no
mask), so no mask bytes move during attention.
Scores are computed transposed ([tk, tq]) so the PV matmul needs no
on-chip transpose of the probabilities; softmax is unnormalized exp with the
denominator from a ones-vector matmul, divided into the attention output.
The attention block loop is software-pipelined: exp(scores) of block j+1
overlaps the PV/denominator accumulation of block j on the PE.
"""

import sys
sys.path.insert(0, '/opt/trn_rl_repo')
sys.path.insert(0, '/opt/trn_rl_repo/concourse')

import numpy as np

S = 2048
D = 4096
HD = 128
FSH = 1024            # features per core (8 heads)
NHL = 8               # heads per core
KT = D // 128         # 32 k-tiles for projections
TSTRIPS = S // 512    # 4 tq strips
NKT = S // 128        # 16 tk tiles
NEG_THRESH = -1.0e8

_cache = {}


def _build(classes, iters=0):
    """Build + compile the per-core Bacc program. classes[j][s] in {0:skip,1:zero,2:add}.

    iters=0: straight-line body (the correctness/grading path).
    iters>=1: wrap the whole body in a hardware For_i loop executing it
    `iters` times — used by test.py to measure the marginal per-iteration
    device time ((wall(N) - wall(1))/(N-1)), which cancels the fixed
    multi-ms axon dispatch overhead that dominates a single execute.
    """
    import contextlib
    import concourse.bacc as bacc
    import concourse.mybir as mybir
    import concourse.tile as tile

    f32 = mybir.dt.float32
    f32r = mybir.dt.float32r
    bf16 = mybir.dt.bfloat16
    EXP = mybir.ActivationFunctionType.Exp

    nc = bacc.Bacc("TRN2", target_bir_lowering=False, debug=False)

    xt_d = nc.dram_tensor("xt", [D, S], bf16, kind="ExternalInput").ap()
    wqt_d = nc.dram_tensor("wqt", [D, FSH], bf16, kind="ExternalInput").ap()
    wkt_d = nc.dram_tensor("wkt", [D, FSH], bf16, kind="ExternalInput").ap()
    wvt_d = nc.dram_tensor("wvt", [D, FSH], bf16, kind="ExternalInput").ap()
    wot_d = nc.dram_tensor("wot", [FSH, D], bf16, kind="ExternalInput").ap()
    cos_d = nc.dram_tensor("cosw", [64, S], bf16, kind="ExternalInput").ap()
    sin_d = nc.dram_tensor("sinw", [64, S], bf16, kind="ExternalInput").ap()
    nsin_d = nc.dram_tensor("nsinw", [64, S], bf16, kind="ExternalInput").ap()
    maskp_d = nc.dram_tensor("maskp", [128, 4 * 512], bf16, kind="ExternalInput").ap()
    on_d = nc.dram_tensor("ones128", [128, 128], bf16, kind="ExternalInput").ap()
    out_d = nc.dram_tensor("out", [S, D], f32, kind="ExternalOutput").ap()

    with tile.TileContext(nc) as tc, \
         nc.allow_low_precision(reason="bf16 data path, fp32 accumulation"), \
         (tc.For_i(0, iters, 1) if iters else contextlib.nullcontext()):
        with tc.tile_pool(name="pdram", bufs=1, space="DRAM") as pdram, \
             tc.tile_pool(name="pconst", bufs=1) as pconst, \
             tc.tile_pool(name="p2h", bufs=2) as p2h:
            qt_d = pdram.tile([FSH, S], bf16, name="qt_spill")
            kt_d = pdram.tile([FSH, S], bf16, name="kt_spill")
            vt_d = pdram.tile([S, FSH], bf16, name="vt_spill")  # v^T: [tokens, feat]
            ones_sb = pconst.tile([128, 128], bf16, name="ones_sb")
            nc.gpsimd.dma_start(out=ones_sb, in_=on_d)
            maskp_sb = pconst.tile([128, 4 * 512], bf16, name="maskp_sb")
            nc.gpsimd.dma_start(out=maskp_sb, in_=maskp_d)
            ones_k = ones_sb[:, 0:1]

            # ---------------- Phase 1: q/k/v projections (+RoPE on q,k) -------------
            # Full x [D, S] (16 MiB bf16) streams into SBUF once; the 24
            # (projection, head) jobs each stream their weight tile once and
            # contract against the resident x. The first W jobs run in
            # wavefront (k-major) order so the PE starts as soon as the first
            # x k-tiles land instead of waiting for the whole stream.
            # Jobs run per-head (v,k,q) so head h's spills complete at job
            # 3h+2 and phase 2's loads (on the scalar queue, idle after the x
            # stream) overlap the phase-1 tail. p2h is allocated BEFORE p1x so
            # its SBUF does not alias the x tiles (no WAR on phase-1 end).
            with tc.tile_pool(name="p1x", bufs=KT) as p1x, \
                 tc.tile_pool(name="p1w", bufs=3) as p1w, \
                 tc.tile_pool(name="p1t", bufs=2) as p1t, \
                 tc.tile_pool(name="p1o", bufs=4) as p1o, \
                 tc.tile_pool(name="p1cs", bufs=1) as p1cs, \
                 tc.tile_pool(name="ps1", bufs=8, space="PSUM") as ps1:
                w_ds = [wqt_d, wkt_d, wvt_d]
                spills = [qt_d, kt_d, vt_d]
                # per-head (k, q, v) so head h's spills are complete early for
                # phase 2's prefetch; k/q lead because the wavefront needs two
                # standard 512-chunk jobs
                jobs = [(proj, i) for i in range(NHL) for proj in (1, 0, 2)]

                def load_w(proj, i):
                    wt = p1w.tile([128, KT, 128], bf16, name="wt")
                    w_ap = w_ds[proj][:, i * 128:(i + 1) * 128].rearrange(
                        "(k p) f -> p k f", p=128)
                    nc.sync.dma_start(out=wt, in_=w_ap)
                    return wt

                W = 2                     # wavefront width (W*4 PSUM banks)

                def load_w_chunked(proj, i):
                    """Weight tile in 4 k-chunks so the wavefront's first
                    matmuls only wait on the first 256 KB, not the full tile."""
                    wt = p1w.tile([128, KT, 128], bf16, name="wt")
                    w_ap = w_ds[proj][:, i * 128:(i + 1) * 128].rearrange(
                        "(k p) f -> p k f", p=128)
                    for g in range(4):
                        ksl = slice(g * (KT // 4), (g + 1) * (KT // 4))
                        nc.sync.dma_start(out=wt[:, ksl, :], in_=w_ap[:, ksl, :])
                    return wt

                wt_wave = [load_w_chunked(*jobs[w]) for w in range(W)]

                xk = []
                for k in range(KT):
                    xt_t = p1x.tile([128, S], bf16, name="xk")
                    nc.scalar.dma_start(out=xt_t, in_=xt_d[k * 128:(k + 1) * 128, :])
                    xk.append(xt_t)
                cos_sb = p1cs.tile([64, S], bf16, name="cos_sb")
                sin_sb = p1cs.tile([64, S], bf16, name="sin_sb")
                nsin_sb = p1cs.tile([64, S], bf16, name="nsin_sb")
                nc.gpsimd.dma_start(out=cos_sb, in_=cos_d)
                nc.gpsimd.dma_start(out=sin_sb, in_=sin_d)
                nc.gpsimd.dma_start(out=nsin_sb, in_=nsin_d)

                def finish_chunk(proj, i, ps, t4):
                    """RoPE (q,k) or copy (v) one [128,512] PSUM chunk and spill."""
                    ot = p1o.tile([128, 512], bf16, name="ot")
                    csl = slice(t4 * 512, (t4 + 1) * 512)
                    if proj < 2:  # RoPE for q, k
                        m1 = p1t.tile([64, 512], f32, name="m1")
                        m2 = p1t.tile([64, 512], f32, name="m2")
                        nc.vector.tensor_mul(m1, ps[0:64], cos_sb[:, csl])
                        nc.vector.tensor_mul(m2, ps[64:128], nsin_sb[:, csl])
                        nc.vector.tensor_add(ot[0:64], m1, m2)
                        m3 = p1t.tile([64, 512], f32, name="m1")
                        m4 = p1t.tile([64, 512], f32, name="m2")
                        nc.vector.tensor_mul(m3, ps[0:64], sin_sb[:, csl])
                        nc.vector.tensor_mul(m4, ps[64:128], cos_sb[:, csl])
                        nc.vector.tensor_add(ot[64:128], m3, m4)
                    else:
                        nc.vector.tensor_copy(ot, ps)
                    nc.gpsimd.dma_start(
                        out=spills[proj][i * 128:(i + 1) * 128,
                                         t4 * 512:(t4 + 1) * 512],
                        in_=ot)

                # wavefront pass: jobs[0:W], k-major
                pss = [[ps1.tile([128, 512], f32, name="ps1") for _ in range(4)]
                       for _ in range(W)]
                for k in range(KT):
                    for w in range(W):
                        for t4 in range(4):
                            nc.tensor.matmul(
                                pss[w][t4], wt_wave[w][:, k, :],
                                xk[k][:, t4 * 512:(t4 + 1) * 512],
                                start=(k == 0), stop=(k == KT - 1))
                wt_next = load_w(*jobs[W])
                for w in range(W):
                    for t4 in range(4):
                        finish_chunk(*jobs[w], pss[w][t4], t4)

                # dense pass: jobs[W:], x fully resident by now
                for idx in range(W, len(jobs)):
                    proj, i = jobs[idx]
                    wt = wt_next
                    if idx + 1 < len(jobs):
                        wt_next = load_w(*jobs[idx + 1])
                    if proj == 2:
                        # v is produced TRANSPOSED ([tokens, hd]) by swapping
                        # the matmul operands (stationary = x token-tile,
                        # moving = wv), so attention needs no on-chip v
                        # transposes at all. 4 token-tiles pack one PSUM bank.
                        for g in range(4):
                            ps = ps1.tile([128, 512], f32, name="ps1")
                            for k in range(KT):
                                for mi in range(4):
                                    m = g * 4 + mi
                                    nc.tensor.matmul(
                                        ps[:, mi * 128:(mi + 1) * 128],
                                        xk[k][:, m * 128:(m + 1) * 128],
                                        wt[:, k, :],
                                        start=(k == 0), stop=(k == KT - 1))
                            ot = p1o.tile([128, 512], bf16, name="ot")
                            nc.vector.tensor_copy(ot, ps)
                            nc.gpsimd.dma_start(
                                out=vt_d[g * 512:(g + 1) * 512,
                                         i * 128:(i + 1) * 128].rearrange(
                                    "(m p) f -> p (m f)", p=128),
                                in_=ot)
                    else:
                        for t4 in range(4):
                            ps = ps1.tile([128, 512], f32, name="ps1")
                            for k in range(KT):
                                nc.tensor.matmul(
                                    ps, wt[:, k, :],
                                    xk[k][:, t4 * 512:(t4 + 1) * 512],
                                    start=(k == 0), stop=(k == KT - 1))
                            finish_chunk(proj, i, ps, t4)

            # ---------------- Phase 2: attention per head ----------------------------
            with tc.tile_pool(name="patt", bufs=1) as patt, \
                 tc.tile_pool(name="p3w", bufs=2) as p3w:
              att_sb = [patt.tile([128, S], bf16, name=f"attT{h}") for h in range(NHL)]
              with tc.tile_pool(name="p2v", bufs=2) as p2v, \
                   tc.tile_pool(name="p2e", bufs=8) as p2e, \
                   tc.tile_pool(name="p2ms", bufs=3) as p2ms, \
                   tc.tile_pool(name="p2r", bufs=4) as p2r, \
                   tc.tile_pool(name="p2o", bufs=4) as p2o, \
                   tc.tile_pool(name="ps2s", bufs=4, space="PSUM") as ps2s, \
                   tc.tile_pool(name="ps2a", bufs=2, space="PSUM") as ps2a, \
                   tc.tile_pool(name="ps2d", bufs=2, space="PSUM") as ps2d:
                  def load_head(h):
                      kt_h = p2h.tile([128, S], bf16, name="kt_h")
                      qt_h = p2h.tile([128, S], bf16, name="qt_h")
                      nc.scalar.dma_start(out=kt_h, in_=kt_d[h * 128:(h + 1) * 128, :])
                      nc.scalar.dma_start(out=qt_h, in_=qt_d[h * 128:(h + 1) * 128, :])
                      # v^T tiles: [tokens-in-tile, hd] per 128-token tile j
                      vb = p2v.tile([128, NKT, 128], bf16, name="vb")
                      nc.scalar.dma_start(
                          out=vb, in_=vt_d[:, h * 128:(h + 1) * 128].rearrange(
                              "(m p) f -> p m f", p=128))
                      return vb, kt_h, qt_h

                  for h in range(NHL):
                      vb, kt_h, qt_h = load_head(h)
                      v_sb = [vb[:, j, :] for j in range(NKT)]
                      for s in range(TSTRIPS):
                          act = [j for j in range(NKT) if classes[j][s] != 0]
                          A = ps2a.tile([128, 512], f32, name="A")
                          Dn = ps2d.tile([1, 512], f32, name="Dn")
                          qs = qt_h[:, s * 512:(s + 1) * 512]

                          def emit_scores(j):
                              sps = ps2s.tile([128, 512], f32, name="sps")
                              nc.tensor.matmul(sps, kt_h[:, j * 128:(j + 1) * 128],
                                               qs, start=True, stop=True)
                              E = p2e.tile([128, 512], bf16, name="E")
                              if classes[j][s] == 2:
                                  p = j - 4 * s
                                  ms = p2ms.tile([128, 512], f32, name="ms")
                                  nc.vector.tensor_add(
                                      ms, sps, maskp_sb[:, p * 512:(p + 1) * 512])
                                  nc.scalar.activation(E, ms, EXP)
                              else:
                                  nc.scalar.activation(E, sps, EXP)
                              return E

                          # 2-deep score/exp pipeline: the exp chain for a
                          # diagonal block (mask add + exp) is ~1.3us, longer
                          # than one block's PE work, so keep two E tiles in
                          # flight ahead of the PV/denominator accumulation.
                          DEPTH = 3
                          Es = [emit_scores(act[d]) for d in range(min(DEPTH, len(act)))]
                          for idx, j in enumerate(act):
                              if idx + DEPTH < len(act):
                                  Es.append(emit_scores(act[idx + DEPTH]))
                              E_cur = Es[idx]
                              first, last = (idx == 0), (idx == len(act) - 1)
                              nc.tensor.matmul(A, v_sb[j], E_cur,
                                               start=first, stop=last)
                              nc.tensor.matmul(Dn, ones_k, E_cur,
                                               start=first, stop=last)
                          rec = p2r.tile([1, 512], f32r, name="rec")
                          nc.vector.reciprocal(rec, Dn[0:1, :])
                          bsb = p2o.tile([128, 512], f32r, name="bsb")
                          nc.gpsimd.partition_broadcast(bsb, rec, 128)
                          nc.vector.tensor_mul(
                              att_sb[h][:, s * 512:(s + 1) * 512], A, bsb)

              # ---------------- Phase 3: output projection ------------------------------
              if True:
                with tc.tile_pool(name="p3o", bufs=6) as p3o, \
                   tc.tile_pool(name="ps3", bufs=8, space="PSUM") as ps3:
                  def load_w3(c):
                      wt = p3w.tile([128, NHL, 512], bf16, name="w3")
                      w_ap = wot_d[:, c * 512:(c + 1) * 512].rearrange(
                          "(k p) f -> p k f", p=128)
                      nc.sync.dma_start(out=wt, in_=w_ap)
                      return wt

                  wt_next3 = load_w3(0)
                  am = att_sb
                  for c in range(8):        # dout chunks of 512
                      wt = wt_next3
                      if c + 1 < 8:
                          wt_next3 = load_w3(c + 1)
                      for m in range(NKT):  # t tiles of 128
                          ps = ps3.tile([128, 512], f32, name="ps3")
                          for k in range(NHL):
                              nc.tensor.matmul(ps, am[k][:, m * 128:(m + 1) * 128],
                                               wt[:, k, :],
                                               start=(k == 0), stop=(k == NHL - 1))
                          ot = p3o.tile([128, 512], f32, name="o3")
                          nc.vector.tensor_copy(ot, ps)
                          nc.gpsimd.dma_start(
                              out=out_d[m * 128:(m + 1) * 128, c * 512:(c + 1) * 512],
                              in_=ot)

    nc.compile()
    return nc


def _host_prep(x, wq, wk, wv, wo, freqs_cos, freqs_sin, mask):
    """Build per-core input maps + mask block classes."""
    import ml_dtypes
    bf16 = ml_dtypes.bfloat16

    x = np.asarray(x, np.float32)
    wq = np.asarray(wq, np.float32)
    wk = np.asarray(wk, np.float32)
    wv = np.asarray(wv, np.float32)
    wo = np.asarray(wo, np.float32)
    mask2 = np.asarray(mask, np.float32).reshape(S, S)

    perm = np.concatenate(
        [hl * 128 + np.concatenate([np.arange(0, 128, 2), np.arange(1, 128, 2)])
         for hl in range(NHL)])
    cosw = np.ascontiguousarray(np.asarray(freqs_cos, np.float32).T)
    sinw = np.ascontiguousarray(np.asarray(freqs_sin, np.float32).T)
    nsinw = np.ascontiguousarray(-sinw)
    maskt = np.ascontiguousarray(mask2.T)

    classes = [[0] * TSTRIPS for _ in range(NKT)]
    for j in range(NKT):
        for s in range(TSTRIPS):
            blk = maskt[j * 128:(j + 1) * 128, s * 512:(s + 1) * 512]
            if (blk <= NEG_THRESH).all():
                classes[j][s] = 0
            elif (blk == 0.0).all():
                classes[j][s] = 1
            else:
                classes[j][s] = 2

    # The partially-masked blocks of a causal mask come in exactly 4 shapes
    # (offset of the 128-row k-block within its 512-col q-strip); keep those
    # resident instead of streaming mask bytes. Verify the assumption holds
    # for the mask we were actually given.
    maskp = np.zeros((128, 4 * 512), np.float32)
    for p in range(4):
        maskp[:, p * 512:(p + 1) * 512] = maskt[p * 128:(p + 1) * 128, 0:512]
    for j in range(NKT):
        for s in range(TSTRIPS):
            if classes[j][s] == 2:
                p = j - 4 * s
                assert 0 <= p < 4, (j, s)
                assert np.array_equal(
                    maskt[j * 128:(j + 1) * 128, s * 512:(s + 1) * 512],
                    maskp[:, p * 512:(p + 1) * 512]), (j, s)

    xts = [np.ascontiguousarray(x[b].T).astype(bf16) for b in range(2)]
    in_maps = []
    for core in range(8):
        b, tp = core // 4, core % 4
        sl = slice(tp * FSH, (tp + 1) * FSH)
        wq_c = wq[sl][perm] * np.float32(1.0 / np.sqrt(HD))
        wk_c = wk[sl][perm]
        in_maps.append({
            "xt": xts[b],
            "wqt": np.ascontiguousarray(wq_c.T).astype(bf16),
            "wkt": np.ascontiguousarray(wk_c.T).astype(bf16),
            "wvt": np.ascontiguousarray(wv[sl].T).astype(bf16),
            "wot": np.ascontiguousarray(wo[:, sl].T).astype(bf16),
            "cosw": cosw.astype(bf16), "sinw": sinw.astype(bf16),
            "nsinw": nsinw.astype(bf16),
            "maskp": maskp.astype(bf16),
            "ones128": np.ones((128, 128), bf16),
        })
    return in_maps, classes


def kernel(x, wq, wk, wv, wo, freqs_cos, freqs_sin, mask, start_pos=0,
           _trace=False):
    from concourse import bass_utils
    in_maps, classes = _host_prep(x, wq, wk, wv, wo, freqs_cos, freqs_sin, mask)
    key = str(classes)
    if key not in _cache:
        _cache[key] = _build(classes)
    nc = _cache[key]
    res = bass_utils.run_bass_kernel_spmd(nc, in_maps, core_ids=list(range(8)),
                                          trace=_trace)
    out = np.zeros((2, S, D), np.float32)
    for core in range(8):
        out[core // 4] += res.results[core]["out"]
    kernel.last_result = res
    return out


if __name__ == "__main__":
    # compile-only smoke test
    classes = [[2 if j * 128 <= s * 512 + 511 and j * 128 + 127 > s * 512 else
                (1 if j * 128 + 127 <= s * 512 else 0)
                for s in range(TSTRIPS)] for j in range(NKT)]
    import time
    t0 = time.time()
    nc = _build(classes)
    print(f"build+bacc-compile: {time.time()-t0:.1f}s")
    from concourse.timeline_sim import TimelineSim
    est = TimelineSim(nc, trace=False).simulate()
    print(f"TimelineSim: {est:.0f} ns")
    if len(sys.argv) > 1 and sys.argv[1] == "neff":
        import tempfile
        from concourse import bass_utils
        t0 = time.time()
        with tempfile.TemporaryDirectory() as td:
            bass_utils.compile_bass_kernel(nc, td)
            print(f"walrus: {time.time()-t0:.1f}s COMPILED OK")


# revision 51
# speedup vs baseline: 1.0382x; 1.0382x over previous
"""Multi-head causal attention (B=2, S=2048, D=4096, H=32, hd=128) on 8 trn2 cores.

Sharding: DP over batch (2) x TP over heads (4 groups of 8 heads).
Core c: batch b = c//4, head-group tp = c%4.
Each core computes a partial output [2048, 4096] (wo row-sharded); host sums
the 4 partials per batch.

Data path is bf16 (inputs rounded host-side; all matmuls accumulate in fp32
PSUM), which halves DMA traffic and lets the full x [4096, 2048] strip stay
resident in SBUF so phase 1 makes a single pass over x with weights streamed
once. Softmax/normalization arithmetic stays fp32.
Host pre-transposes x / weights so every DMA is natural-layout.
q/k head dims are de-interleaved (evens then odds) on the host so RoPE becomes
full-tile DVE ops on partition halves; the permutation is consistent between
q and k so scores are unchanged. v / wo stay in natural order.
The causal mask enters as 4 distinct [128, 512] diagonal-block patterns kept
resident in SBUF (fully-masked blocks are skipped, fully-open blocks need no
mask), so no mask bytes move during attention.
v is produced already TRANSPOSED in phase 1 (stationary = x token-tile,
moving = wv; same PE cost either way) so attention needs no on-chip v
transposes; weights are pre-swizzled host-side to [NHL,128,KT,128] so every
weight DMA is one fully-contiguous per-head tile.
Scores are computed transposed ([tk, tq]) so the PV matmul needs no
on-chip transpose of the probabilities; softmax is unnormalized exp with the
denominator from a ones-vector matmul, divided into the attention output.
The attention block loop is software-pipelined 3 deep: exp(scores) of blocks
j+1..j+3 overlap the PV/denominator accumulation of block j on the PE.
"""

import sys
sys.path.insert(0, '/opt/trn_rl_repo')
sys.path.insert(0, '/opt/trn_rl_repo/concourse')

import numpy as np

S = 2048
D = 4096
HD = 128
FSH = 1024            # features per core (8 heads)
NHL = 8               # heads per core
KT = D // 128         # 32 k-tiles for projections
TSTRIPS = S // 512    # 4 tq strips
NKT = S // 128        # 16 tk tiles
NEG_THRESH = -1.0e8

_cache = {}


def _build(classes, iters=0, phases=(1, 2, 3), depth=3, xsplit=False, pair_strips=False):
    """Build + compile the per-core Bacc program. classes[j][s] in {0:skip,1:zero,2:add}.

    iters=0: straight-line body (the correctness/grading path).
    iters>=1: wrap the whole body in a hardware For_i loop executing it
    `iters` times — used by test.py to measure the marginal per-iteration
    device time ((wall(N) - wall(1))/(N-1)), which cancels the fixed
    multi-ms axon dispatch overhead that dominates a single execute.
    """
    import contextlib
    import concourse.bacc as bacc
    import concourse.mybir as mybir
    import concourse.tile as tile

    f32 = mybir.dt.float32
    f32r = mybir.dt.float32r
    bf16 = mybir.dt.bfloat16
    EXP = mybir.ActivationFunctionType.Exp

    nc = bacc.Bacc("TRN2", target_bir_lowering=False, debug=False)

    xt_d = nc.dram_tensor("xt", [D, S], bf16, kind="ExternalInput").ap()
    wqt_d = nc.dram_tensor("wqt", [NHL, 128, KT, 128], bf16, kind="ExternalInput").ap()
    wkt_d = nc.dram_tensor("wkt", [NHL, 128, KT, 128], bf16, kind="ExternalInput").ap()
    wvt_d = nc.dram_tensor("wvt", [NHL, 128, KT, 128], bf16, kind="ExternalInput").ap()
    wot_d = nc.dram_tensor("wot", [FSH, D], bf16, kind="ExternalInput").ap()
    cos_d = nc.dram_tensor("cosw", [64, S], bf16, kind="ExternalInput").ap()
    sin_d = nc.dram_tensor("sinw", [64, S], bf16, kind="ExternalInput").ap()
    nsin_d = nc.dram_tensor("nsinw", [64, S], bf16, kind="ExternalInput").ap()
    maskp_d = nc.dram_tensor("maskp", [128, 4 * 512], bf16, kind="ExternalInput").ap()
    on_d = nc.dram_tensor("ones128", [128, 128], bf16, kind="ExternalInput").ap()
    out_d = nc.dram_tensor("out", [S, D], f32, kind="ExternalOutput").ap()

    with tile.TileContext(nc) as tc, \
         nc.allow_low_precision(reason="bf16 data path, fp32 accumulation"), \
         (tc.For_i(0, iters, 1) if iters else contextlib.nullcontext()):
        with tc.tile_pool(name="pdram", bufs=1, space="DRAM") as pdram, \
             tc.tile_pool(name="pconst", bufs=1) as pconst, \
             tc.tile_pool(name="p2h", bufs=2) as p2h:
            qt_d = pdram.tile([FSH, S], bf16, name="qt_spill")
            kt_d = pdram.tile([FSH, S], bf16, name="kt_spill")
            vt_d = pdram.tile([NHL, 128, NKT, 128], bf16, name="vt_spill")  # v^T per head: [p, tok-tile, hd]
            ones_sb = pconst.tile([128, 128], bf16, name="ones_sb")
            nc.gpsimd.dma_start(out=ones_sb, in_=on_d)
            maskp_sb = pconst.tile([128, 4 * 512], bf16, name="maskp_sb")
            nc.gpsimd.dma_start(out=maskp_sb, in_=maskp_d)
            ones_k = ones_sb[:, 0:1]

            # ---------------- Phase 1: q/k/v projections (+RoPE on q,k) -------------
            # Full x [D, S] (16 MiB bf16) streams into SBUF once; the 24
            # (projection, head) jobs each stream their weight tile once and
            # contract against the resident x. The first W jobs run in
            # wavefront (k-major) order so the PE starts as soon as the first
            # x k-tiles land instead of waiting for the whole stream.
            # Jobs run per-head (v,k,q) so head h's spills complete at job
            # 3h+2 and phase 2's loads (on the scalar queue, idle after the x
            # stream) overlap the phase-1 tail. p2h is allocated BEFORE p1x so
            # its SBUF does not alias the x tiles (no WAR on phase-1 end).
            with tc.tile_pool(name="p1x", bufs=KT) as p1x, \
                 tc.tile_pool(name="p1w", bufs=3) as p1w, \
                 tc.tile_pool(name="p1t", bufs=2) as p1t, \
                 tc.tile_pool(name="p1o", bufs=4) as p1o, \
                 tc.tile_pool(name="p1cs", bufs=1) as p1cs, \
                 tc.tile_pool(name="ps1", bufs=8, space="PSUM") as ps1:
                w_ds = [wqt_d, wkt_d, wvt_d]
                spills = [qt_d, kt_d, vt_d]
                # per-head (k, q, v) so head h's spills are complete early for
                # phase 2's prefetch; k/q lead because the wavefront needs two
                # standard 512-chunk jobs
                jobs = [(proj, i) for i in range(NHL) for proj in (1, 0, 2)]

                def load_w(proj, i):
                    wt = p1w.tile([128, KT, 128], bf16, name="wt")
                    nc.sync.dma_start(out=wt, in_=w_ds[proj][i])
                    return wt

                W = 2                     # wavefront width (W*4 PSUM banks)

                def load_w_chunked(proj, i):
                    """Weight tile in 4 k-chunks so the wavefront's first
                    matmuls only wait on the first 256 KB, not the full tile."""
                    wt = p1w.tile([128, KT, 128], bf16, name="wt")
                    for g in range(4):
                        ksl = slice(g * (KT // 4), (g + 1) * (KT // 4))
                        nc.sync.dma_start(out=wt[:, ksl, :], in_=w_ds[proj][i][:, ksl, :])
                    return wt

                wt_wave = [load_w_chunked(*jobs[w]) for w in range(W)]

                xk = []
                for k in range(KT):
                    xt_t = p1x.tile([128, S], bf16, name="xk")
                    eng = nc.scalar if (not xsplit or k % 2 == 0) else nc.sync
                    eng.dma_start(out=xt_t, in_=xt_d[k * 128:(k + 1) * 128, :])
                    xk.append(xt_t)
                cos_sb = p1cs.tile([64, S], bf16, name="cos_sb")
                sin_sb = p1cs.tile([64, S], bf16, name="sin_sb")
                nsin_sb = p1cs.tile([64, S], bf16, name="nsin_sb")
                nc.gpsimd.dma_start(out=cos_sb, in_=cos_d)
                nc.gpsimd.dma_start(out=sin_sb, in_=sin_d)
                nc.gpsimd.dma_start(out=nsin_sb, in_=nsin_d)

                def finish_chunk(proj, i, ps, t4):
                    """RoPE (q,k) or copy (v) one [128,512] PSUM chunk and spill."""
                    ot = p1o.tile([128, 512], bf16, name="ot")
                    csl = slice(t4 * 512, (t4 + 1) * 512)
                    if proj < 2:  # RoPE for q, k
                        m1 = p1t.tile([64, 512], f32, name="m1")
                        m2 = p1t.tile([64, 512], f32, name="m2")
                        nc.vector.tensor_mul(m1, ps[0:64], cos_sb[:, csl])
                        nc.vector.tensor_mul(m2, ps[64:128], nsin_sb[:, csl])
                        nc.vector.tensor_add(ot[0:64], m1, m2)
                        m3 = p1t.tile([64, 512], f32, name="m1")
                        m4 = p1t.tile([64, 512], f32, name="m2")
                        nc.vector.tensor_mul(m3, ps[0:64], sin_sb[:, csl])
                        nc.vector.tensor_mul(m4, ps[64:128], cos_sb[:, csl])
                        nc.vector.tensor_add(ot[64:128], m3, m4)
                    else:
                        nc.vector.tensor_copy(ot, ps)
                    nc.gpsimd.dma_start(
                        out=spills[proj][i * 128:(i + 1) * 128,
                                         t4 * 512:(t4 + 1) * 512],
                        in_=ot)

                # wavefront pass: jobs[0:W], k-major
                pss = [[ps1.tile([128, 512], f32, name="ps1") for _ in range(4)]
                       for _ in range(W)]
                for k in range(KT):
                    for w in range(W):
                        for t4 in range(4):
                            nc.tensor.matmul(
                                pss[w][t4], wt_wave[w][:, k, :],
                                xk[k][:, t4 * 512:(t4 + 1) * 512],
                                start=(k == 0), stop=(k == KT - 1))
                wt_next = load_w(*jobs[W])
                for w in range(W):
                    for t4 in range(4):
                        finish_chunk(*jobs[w], pss[w][t4], t4)

                # dense pass: jobs[W:], x fully resident by now
                for idx in range(W, len(jobs)):
                    proj, i = jobs[idx]
                    wt = wt_next
                    if idx + 1 < len(jobs):
                        wt_next = load_w(*jobs[idx + 1])
                    if proj == 2:
                        # v is produced TRANSPOSED ([tokens, hd]) by swapping
                        # the matmul operands (stationary = x token-tile,
                        # moving = wv), so attention needs no on-chip v
                        # transposes at all. Each 128-token accumulation group
                        # owns a full PSUM bank (start zeroes the whole 2KB
                        # zero region, so groups cannot share a bank).
                        for g in range(4):
                            otv = p1o.tile([128, 4, 128], bf16, name="otv")
                            for mi in range(4):
                                m = g * 4 + mi
                                ps = ps1.tile([128, 512], f32, name="ps1")
                                for k in range(KT):
                                    nc.tensor.matmul(
                                        ps[:, 0:128],
                                        xk[k][:, m * 128:(m + 1) * 128],
                                        wt[:, k, :],
                                        start=(k == 0), stop=(k == KT - 1))
                                nc.vector.tensor_copy(otv[:, mi, :], ps[:, 0:128])
                            nc.gpsimd.dma_start(
                                out=vt_d[i][:, g * 4:(g + 1) * 4, :],
                                in_=otv)
                    else:
                        for t4 in range(4):
                            ps = ps1.tile([128, 512], f32, name="ps1")
                            for k in range(KT):
                                nc.tensor.matmul(
                                    ps, wt[:, k, :],
                                    xk[k][:, t4 * 512:(t4 + 1) * 512],
                                    start=(k == 0), stop=(k == KT - 1))
                            finish_chunk(proj, i, ps, t4)

            # ---------------- Phase 2: attention per head ----------------------------
            if 2 in phases:
             with tc.tile_pool(name="patt", bufs=1) as patt, \
                  tc.tile_pool(name="p3w", bufs=2) as p3w:
              att_sb = [patt.tile([128, S], bf16, name=f"attT{h}") for h in range(NHL)]
              with tc.tile_pool(name="p2v", bufs=2) as p2v, \
                   tc.tile_pool(name="p2e", bufs=8) as p2e, \
                   tc.tile_pool(name="p2ms", bufs=3) as p2ms, \
                   tc.tile_pool(name="p2r", bufs=4) as p2r, \
                   tc.tile_pool(name="p2o", bufs=4) as p2o, \
                   tc.tile_pool(name="ps2s", bufs=4, space="PSUM") as ps2s, \
                   tc.tile_pool(name="ps2a", bufs=2, space="PSUM") as ps2a, \
                   tc.tile_pool(name="ps2d", bufs=2, space="PSUM") as ps2d:
                  def load_head(h):
                      kt_h = p2h.tile([128, S], bf16, name="kt_h")
                      qt_h = p2h.tile([128, S], bf16, name="qt_h")
                      nc.scalar.dma_start(out=kt_h, in_=kt_d[h * 128:(h + 1) * 128, :])
                      nc.scalar.dma_start(out=qt_h, in_=qt_d[h * 128:(h + 1) * 128, :])
                      # v^T tiles: [tokens-in-tile, hd] per 128-token tile j
                      vb = p2v.tile([128, NKT, 128], bf16, name="vb")
                      nc.scalar.dma_start(out=vb, in_=vt_d[h])
                      return vb, kt_h, qt_h

                  for h in range(NHL):
                      vb, kt_h, qt_h = load_head(h)
                      v_sb = [vb[:, j, :] for j in range(NKT)]

                      def make_strip(s):
                          c = {
                              "s": s,
                              "act": [j for j in range(NKT) if classes[j][s] != 0],
                              "A": ps2a.tile([128, 512], f32, name="A"),
                              "Dn": ps2d.tile([1, 512], f32, name="Dn"),
                              "qs": qt_h[:, s * 512:(s + 1) * 512],
                              "Es": [], "idx": 0,
                          }
                          return c

                      def emit_scores(c, i):
                          s, j = c["s"], c["act"][i]
                          sps = ps2s.tile([128, 512], f32, name="sps")
                          nc.tensor.matmul(sps, kt_h[:, j * 128:(j + 1) * 128],
                                           c["qs"], start=True, stop=True)
                          E = p2e.tile([128, 512], bf16, name="E")
                          if classes[j][s] == 2:
                              p = j - 4 * s
                              ms = p2ms.tile([128, 512], f32, name="ms")
                              nc.vector.tensor_add(
                                  ms, sps, maskp_sb[:, p * 512:(p + 1) * 512])
                              nc.scalar.activation(E, ms, EXP)
                          else:
                              nc.scalar.activation(E, sps, EXP)
                          c["Es"].append(E)

                      def step(c):
                          """One PV/denominator accumulation step of strip c."""
                          i = c["idx"]
                          if i + depth < len(c["act"]):
                              emit_scores(c, i + depth)
                          j = c["act"][i]
                          first, last = (i == 0), (i == len(c["act"]) - 1)
                          nc.tensor.matmul(c["A"], v_sb[j], c["Es"][i],
                                           start=first, stop=last)
                          nc.tensor.matmul(c["Dn"], ones_k, c["Es"][i],
                                           start=first, stop=last)
                          c["idx"] += 1

                      def normalize(c):
                          rec = p2r.tile([1, 512], f32r, name="rec")
                          nc.vector.reciprocal(rec, c["Dn"][0:1, :])
                          bsb = p2o.tile([128, 512], f32r, name="bsb")
                          nc.gpsimd.partition_broadcast(bsb, rec, 128)
                          s = c["s"]
                          nc.vector.tensor_mul(
                              att_sb[h][:, s * 512:(s + 1) * 512], c["A"], bsb)

                      if not pair_strips:
                          # depth-deep score/exp pipeline: the exp chain for a
                          # diagonal block (mask add + exp) is longer than one
                          # block's PE work, so keep E tiles in flight ahead
                          # of the PV/denominator accumulation.
                          for s in range(TSTRIPS):
                              c = make_strip(s)
                              for d in range(min(depth, len(c["act"]))):
                                  emit_scores(c, d)
                              while c["idx"] < len(c["act"]):
                                  step(c)
                              normalize(c)
                      else:
                          # process strips in pairs: two independent A/Dn
                          # accumulation chains give the PE twice the slack
                          # against the exp-chain latency
                          for sp in range(0, TSTRIPS, 2):
                              cs = [make_strip(sp), make_strip(sp + 1)]
                              for c in cs:
                                  for d in range(min(depth, len(c["act"]))):
                                      emit_scores(c, d)
                              while any(c["idx"] < len(c["act"]) for c in cs):
                                  for c in cs:
                                      if c["idx"] < len(c["act"]):
                                          step(c)
                              for c in cs:
                                  normalize(c)

              # ---------------- Phase 3: output projection ------------------------------
              if 3 in phases:
                with tc.tile_pool(name="p3o", bufs=6) as p3o, \
                   tc.tile_pool(name="ps3", bufs=8, space="PSUM") as ps3:
                  def load_w3(c):
                      wt = p3w.tile([128, NHL, 512], bf16, name="w3")
                      w_ap = wot_d[:, c * 512:(c + 1) * 512].rearrange(
                          "(k p) f -> p k f", p=128)
                      nc.sync.dma_start(out=wt, in_=w_ap)
                      return wt

                  wt_next3 = load_w3(0)
                  am = att_sb
                  for c in range(8):        # dout chunks of 512
                      wt = wt_next3
                      if c + 1 < 8:
                          wt_next3 = load_w3(c + 1)
                      for m in range(NKT):  # t tiles of 128
                          ps = ps3.tile([128, 512], f32, name="ps3")
                          for k in range(NHL):
                              nc.tensor.matmul(ps, am[k][:, m * 128:(m + 1) * 128],
                                               wt[:, k, :],
                                               start=(k == 0), stop=(k == NHL - 1))
                          ot = p3o.tile([128, 512], f32, name="o3")
                          nc.vector.tensor_copy(ot, ps)
                          nc.gpsimd.dma_start(
                              out=out_d[m * 128:(m + 1) * 128, c * 512:(c + 1) * 512],
                              in_=ot)

    nc.compile()
    return nc


def _host_prep(x, wq, wk, wv, wo, freqs_cos, freqs_sin, mask):
    """Build per-core input maps + mask block classes."""
    import ml_dtypes
    bf16 = ml_dtypes.bfloat16

    x = np.asarray(x, np.float32)
    wq = np.asarray(wq, np.float32)
    wk = np.asarray(wk, np.float32)
    wv = np.asarray(wv, np.float32)
    wo = np.asarray(wo, np.float32)
    mask2 = np.asarray(mask, np.float32).reshape(S, S)

    perm = np.concatenate(
        [hl * 128 + np.concatenate([np.arange(0, 128, 2), np.arange(1, 128, 2)])
         for hl in range(NHL)])
    cosw = np.ascontiguousarray(np.asarray(freqs_cos, np.float32).T)
    sinw = np.ascontiguousarray(np.asarray(freqs_sin, np.float32).T)
    nsinw = np.ascontiguousarray(-sinw)
    maskt = np.ascontiguousarray(mask2.T)

    classes = [[0] * TSTRIPS for _ in range(NKT)]
    for j in range(NKT):
        for s in range(TSTRIPS):
            blk = maskt[j * 128:(j + 1) * 128, s * 512:(s + 1) * 512]
            if (blk <= NEG_THRESH).all():
                classes[j][s] = 0
            elif (blk == 0.0).all():
                classes[j][s] = 1
            else:
                classes[j][s] = 2

    # The partially-masked blocks of a causal mask come in exactly 4 shapes
    # (offset of the 128-row k-block within its 512-col q-strip); keep those
    # resident instead of streaming mask bytes. Verify the assumption holds
    # for the mask we were actually given.
    maskp = np.zeros((128, 4 * 512), np.float32)
    for p in range(4):
        maskp[:, p * 512:(p + 1) * 512] = maskt[p * 128:(p + 1) * 128, 0:512]
    for j in range(NKT):
        for s in range(TSTRIPS):
            if classes[j][s] == 2:
                p = j - 4 * s
                assert 0 <= p < 4, (j, s)
                assert np.array_equal(
                    maskt[j * 128:(j + 1) * 128, s * 512:(s + 1) * 512],
                    maskp[:, p * 512:(p + 1) * 512]), (j, s)

    xts = [np.ascontiguousarray(x[b].T).astype(bf16) for b in range(2)]
    in_maps = []
    for core in range(8):
        b, tp = core // 4, core % 4
        sl = slice(tp * FSH, (tp + 1) * FSH)
        wq_c = wq[sl][perm] * np.float32(1.0 / np.sqrt(HD))
        wk_c = wk[sl][perm]

        def swz(w_c):
            # [D, FSH] (transposed weight) -> [NHL, 128, KT, 128] so each
            # per-head tile loads as one fully-contiguous DMA
            return np.ascontiguousarray(
                w_c.T.reshape(KT, 128, NHL, 128).transpose(2, 1, 0, 3)).astype(bf16)

        in_maps.append({
            "xt": xts[b],
            "wqt": swz(wq_c),
            "wkt": swz(wk_c),
            "wvt": swz(wv[sl]),
            "wot": np.ascontiguousarray(wo[:, sl].T).astype(bf16),
            "cosw": cosw.astype(bf16), "sinw": sinw.astype(bf16),
            "nsinw": nsinw.astype(bf16),
            "maskp": maskp.astype(bf16),
            "ones128": np.ones((128, 128), bf16),
        })
    return in_maps, classes


def kernel(x, wq, wk, wv, wo, freqs_cos, freqs_sin, mask, start_pos=0,
           _trace=False):
    from concourse import bass_utils
    in_maps, classes = _host_prep(x, wq, wk, wv, wo, freqs_cos, freqs_sin, mask)
    key = str(classes)
    if key not in _cache:
        _cache[key] = _build(classes)
    nc = _cache[key]
    res = bass_utils.run_bass_kernel_spmd(nc, in_maps, core_ids=list(range(8)),
                                          trace=_trace)
    out = np.zeros((2, S, D), np.float32)
    for core in range(8):
        out[core // 4] += res.results[core]["out"]
    kernel.last_result = res
    return out


if __name__ == "__main__":
    # compile-only smoke test
    classes = [[2 if j * 128 <= s * 512 + 511 and j * 128 + 127 > s * 512 else
                (1 if j * 128 + 127 <= s * 512 else 0)
                for s in range(TSTRIPS)] for j in range(NKT)]
    import time
    t0 = time.time()
    nc = _build(classes)
    print(f"build+bacc-compile: {time.time()-t0:.1f}s")
    from concourse.timeline_sim import TimelineSim
    est = TimelineSim(nc, trace=False).simulate()
    print(f"TimelineSim: {est:.0f} ns")
    if len(sys.argv) > 1 and sys.argv[1] == "neff":
        import tempfile
        from concourse import bass_utils
        t0 = time.time()
        with tempfile.TemporaryDirectory() as td:
            bass_utils.compile_bass_kernel(nc, td)
            print(f"walrus: {time.time()-t0:.1f}s COMPILED OK")


# revision 53
# speedup vs baseline: 1.0537x; 1.0149x over previous
"""Multi-head causal attention (B=2, S=2048, D=4096, H=32, hd=128) on 8 trn2 cores.

Sharding: DP over batch (2) x TP over heads (4 groups of 8 heads).
Core c: batch b = c//4, head-group tp = c%4.
Each core computes a partial output [2048, 4096] (wo row-sharded); host sums
the 4 partials per batch.

Data path is bf16 (inputs rounded host-side; all matmuls accumulate in fp32
PSUM), which halves DMA traffic and lets the full x [4096, 2048] strip stay
resident in SBUF so phase 1 makes a single pass over x with weights streamed
once. Softmax/normalization arithmetic stays fp32.
Host pre-transposes x / weights so every DMA is natural-layout.
q/k head dims are de-interleaved (evens then odds) on the host so RoPE becomes
full-tile DVE ops on partition halves; the permutation is consistent between
q and k so scores are unchanged. v / wo stay in natural order.
The causal mask enters as 4 distinct [128, 512] diagonal-block patterns kept
resident in SBUF (fully-masked blocks are skipped, fully-open blocks need no
mask), so no mask bytes move during attention.
v is produced already TRANSPOSED in phase 1 (stationary = x token-tile,
moving = wv; same PE cost either way) so attention needs no on-chip v
transposes; weights are pre-swizzled host-side to [NHL,128,KT,128] so every
weight DMA is one fully-contiguous per-head tile.
Scores are computed transposed ([tk, tq]) so the PV matmul needs no
on-chip transpose of the probabilities; softmax is unnormalized exp with the
denominator from a ones-vector matmul, divided into the attention output.
The attention block loop is software-pipelined 3 deep: exp(scores) of blocks
j+1..j+3 overlap the PV/denominator accumulation of block j on the PE.
Spills and output stores are batched 4 chunks per DMA (96->24 spill DMAs,
128->32 output DMAs) and PSUM evacuation copies run on the otherwise-idle
Activation engine — on this hardware, per-instruction/semaphore count costs
real time that the cost model does not capture.
"""

import sys
sys.path.insert(0, '/opt/trn_rl_repo')
sys.path.insert(0, '/opt/trn_rl_repo/concourse')

import numpy as np

S = 2048
D = 4096
HD = 128
FSH = 1024            # features per core (8 heads)
NHL = 8               # heads per core
KT = D // 128         # 32 k-tiles for projections
TSTRIPS = S // 512    # 4 tq strips
NKT = S // 128        # 16 tk tiles
NEG_THRESH = -1.0e8

_cache = {}


def _build(classes, iters=0, phases=(1, 2, 3), depth=3, xsplit=False, pair_strips=False):
    """Build + compile the per-core Bacc program. classes[j][s] in {0:skip,1:zero,2:add}.

    iters=0: straight-line body (the correctness/grading path).
    iters>=1: wrap the whole body in a hardware For_i loop executing it
    `iters` times — used by test.py to measure the marginal per-iteration
    device time ((wall(N) - wall(1))/(N-1)), which cancels the fixed
    multi-ms axon dispatch overhead that dominates a single execute.
    """
    import contextlib
    import concourse.bacc as bacc
    import concourse.mybir as mybir
    import concourse.tile as tile

    f32 = mybir.dt.float32
    f32r = mybir.dt.float32r
    bf16 = mybir.dt.bfloat16
    EXP = mybir.ActivationFunctionType.Exp
    COPY = mybir.ActivationFunctionType.Copy

    nc = bacc.Bacc("TRN2", target_bir_lowering=False, debug=False)

    xt_d = nc.dram_tensor("xt", [D, S], bf16, kind="ExternalInput").ap()
    wqt_d = nc.dram_tensor("wqt", [NHL, 128, KT, 128], bf16, kind="ExternalInput").ap()
    wkt_d = nc.dram_tensor("wkt", [NHL, 128, KT, 128], bf16, kind="ExternalInput").ap()
    wvt_d = nc.dram_tensor("wvt", [NHL, 128, KT, 128], bf16, kind="ExternalInput").ap()
    wot_d = nc.dram_tensor("wot", [FSH, D], bf16, kind="ExternalInput").ap()
    cos_d = nc.dram_tensor("cosw", [64, S], bf16, kind="ExternalInput").ap()
    sin_d = nc.dram_tensor("sinw", [64, S], bf16, kind="ExternalInput").ap()
    nsin_d = nc.dram_tensor("nsinw", [64, S], bf16, kind="ExternalInput").ap()
    maskp_d = nc.dram_tensor("maskp", [128, 4 * 512], bf16, kind="ExternalInput").ap()
    on_d = nc.dram_tensor("ones128", [128, 128], bf16, kind="ExternalInput").ap()
    out_d = nc.dram_tensor("out", [S, D], f32, kind="ExternalOutput").ap()

    with tile.TileContext(nc) as tc, \
         nc.allow_low_precision(reason="bf16 data path, fp32 accumulation"), \
         (tc.For_i(0, iters, 1) if iters else contextlib.nullcontext()):
        with tc.tile_pool(name="pdram", bufs=1, space="DRAM") as pdram, \
             tc.tile_pool(name="pconst", bufs=1) as pconst, \
             tc.tile_pool(name="p2h", bufs=2) as p2h:
            qt_d = pdram.tile([FSH, S], bf16, name="qt_spill")
            kt_d = pdram.tile([FSH, S], bf16, name="kt_spill")
            vt_d = pdram.tile([NHL, 128, NKT, 128], bf16, name="vt_spill")  # v^T per head: [p, tok-tile, hd]
            ones_sb = pconst.tile([128, 128], bf16, name="ones_sb")
            nc.gpsimd.dma_start(out=ones_sb, in_=on_d)
            maskp_sb = pconst.tile([128, 4 * 512], bf16, name="maskp_sb")
            nc.gpsimd.dma_start(out=maskp_sb, in_=maskp_d)
            ones_k = ones_sb[:, 0:1]

            # ---------------- Phase 1: q/k/v projections (+RoPE on q,k) -------------
            # Full x [D, S] (16 MiB bf16) streams into SBUF once; the 24
            # (projection, head) jobs each stream their weight tile once and
            # contract against the resident x. The first W jobs run in
            # wavefront (k-major) order so the PE starts as soon as the first
            # x k-tiles land instead of waiting for the whole stream.
            # Jobs run per-head (v,k,q) so head h's spills complete at job
            # 3h+2 and phase 2's loads (on the scalar queue, idle after the x
            # stream) overlap the phase-1 tail. p2h is allocated BEFORE p1x so
            # its SBUF does not alias the x tiles (no WAR on phase-1 end).
            with tc.tile_pool(name="p1x", bufs=KT) as p1x, \
                 tc.tile_pool(name="p1w", bufs=3) as p1w, \
                 tc.tile_pool(name="p1t", bufs=2) as p1t, \
                 tc.tile_pool(name="p1o", bufs=2) as p1o, \
                 tc.tile_pool(name="p1cs", bufs=1) as p1cs, \
                 tc.tile_pool(name="ps1", bufs=8, space="PSUM") as ps1:
                w_ds = [wqt_d, wkt_d, wvt_d]
                spills = [qt_d, kt_d, vt_d]
                # per-head (k, q, v) so head h's spills are complete early for
                # phase 2's prefetch; k/q lead because the wavefront needs two
                # standard 512-chunk jobs
                jobs = [(proj, i) for i in range(NHL) for proj in (1, 0, 2)]

                def load_w(proj, i):
                    wt = p1w.tile([128, KT, 128], bf16, name="wt")
                    nc.sync.dma_start(out=wt, in_=w_ds[proj][i])
                    return wt

                W = 2                     # wavefront width (W*4 PSUM banks)

                def load_w_chunked(proj, i):
                    """Weight tile in 4 k-chunks so the wavefront's first
                    matmuls only wait on the first 256 KB, not the full tile."""
                    wt = p1w.tile([128, KT, 128], bf16, name="wt")
                    for g in range(4):
                        ksl = slice(g * (KT // 4), (g + 1) * (KT // 4))
                        nc.sync.dma_start(out=wt[:, ksl, :], in_=w_ds[proj][i][:, ksl, :])
                    return wt

                wt_wave = [load_w_chunked(*jobs[w]) for w in range(W)]

                xk = []
                for k in range(KT):
                    xt_t = p1x.tile([128, S], bf16, name="xk")
                    eng = nc.scalar if (not xsplit or k % 2 == 0) else nc.sync
                    eng.dma_start(out=xt_t, in_=xt_d[k * 128:(k + 1) * 128, :])
                    xk.append(xt_t)
                cos_sb = p1cs.tile([64, S], bf16, name="cos_sb")
                sin_sb = p1cs.tile([64, S], bf16, name="sin_sb")
                nsin_sb = p1cs.tile([64, S], bf16, name="nsin_sb")
                nc.gpsimd.dma_start(out=cos_sb, in_=cos_d)
                nc.gpsimd.dma_start(out=sin_sb, in_=sin_d)
                nc.gpsimd.dma_start(out=nsin_sb, in_=nsin_d)

                def rope_chunk(stage, ps, t4):
                    """RoPE one [128,512] PSUM chunk into stage[:, t4, :]."""
                    csl = slice(t4 * 512, (t4 + 1) * 512)
                    m1 = p1t.tile([64, 512], f32, name="m1")
                    m2 = p1t.tile([64, 512], f32, name="m2")
                    nc.vector.tensor_mul(m1, ps[0:64], cos_sb[:, csl])
                    nc.vector.tensor_mul(m2, ps[64:128], nsin_sb[:, csl])
                    nc.vector.tensor_add(stage[0:64, t4, :], m1, m2)
                    m3 = p1t.tile([64, 512], f32, name="m1")
                    m4 = p1t.tile([64, 512], f32, name="m2")
                    nc.vector.tensor_mul(m3, ps[0:64], sin_sb[:, csl])
                    nc.vector.tensor_mul(m4, ps[64:128], cos_sb[:, csl])
                    nc.vector.tensor_add(stage[64:128, t4, :], m3, m4)

                def spill_job(proj, i, stage):
                    """One batched spill DMA per (projection, head) job."""
                    nc.gpsimd.dma_start(
                        out=spills[proj][i * 128:(i + 1) * 128, :].rearrange(
                            "p (m f) -> p m f", m=4),
                        in_=stage)

                # wavefront pass: jobs[0:W], k-major
                pss = [[ps1.tile([128, 512], f32, name="ps1") for _ in range(4)]
                       for _ in range(W)]
                for k in range(KT):
                    for w in range(W):
                        for t4 in range(4):
                            nc.tensor.matmul(
                                pss[w][t4], wt_wave[w][:, k, :],
                                xk[k][:, t4 * 512:(t4 + 1) * 512],
                                start=(k == 0), stop=(k == KT - 1))
                wt_next = load_w(*jobs[W])
                for w in range(W):
                    stage = p1o.tile([128, 4, 512], bf16, name="stq")
                    for t4 in range(4):
                        rope_chunk(stage, pss[w][t4], t4)
                    spill_job(*jobs[w], stage)

                # dense pass: jobs[W:], x fully resident by now
                for idx in range(W, len(jobs)):
                    proj, i = jobs[idx]
                    wt = wt_next
                    if idx + 1 < len(jobs):
                        wt_next = load_w(*jobs[idx + 1])
                    if proj == 2:
                        # v is produced TRANSPOSED ([tokens, hd]) by swapping
                        # the matmul operands (stationary = x token-tile,
                        # moving = wv), so attention needs no on-chip v
                        # transposes at all. Each 128-token accumulation group
                        # owns a full PSUM bank (start zeroes the whole 2KB
                        # zero region, so groups cannot share a bank).
                        for g in range(4):
                            otv = p1o.tile([128, 4, 128], bf16, name="otv")
                            for mi in range(4):
                                m = g * 4 + mi
                                ps = ps1.tile([128, 512], f32, name="ps1")
                                for k in range(KT):
                                    nc.tensor.matmul(
                                        ps[:, 0:128],
                                        xk[k][:, m * 128:(m + 1) * 128],
                                        wt[:, k, :],
                                        start=(k == 0), stop=(k == KT - 1))
                                nc.scalar.activation(otv[:, mi, :], ps[:, 0:128], COPY)
                            nc.gpsimd.dma_start(
                                out=vt_d[i][:, g * 4:(g + 1) * 4, :],
                                in_=otv)
                    else:
                        stage = p1o.tile([128, 4, 512], bf16, name="stq")
                        for t4 in range(4):
                            ps = ps1.tile([128, 512], f32, name="ps1")
                            for k in range(KT):
                                nc.tensor.matmul(
                                    ps, wt[:, k, :],
                                    xk[k][:, t4 * 512:(t4 + 1) * 512],
                                    start=(k == 0), stop=(k == KT - 1))
                            rope_chunk(stage, ps, t4)
                        spill_job(proj, i, stage)

            # ---------------- Phase 2: attention per head ----------------------------
            if 2 in phases:
             with tc.tile_pool(name="patt", bufs=1) as patt, \
                  tc.tile_pool(name="p3w", bufs=2) as p3w:
              att_sb = [patt.tile([128, S], bf16, name=f"attT{h}") for h in range(NHL)]
              with tc.tile_pool(name="p2v", bufs=2) as p2v, \
                   tc.tile_pool(name="p2e", bufs=8) as p2e, \
                   tc.tile_pool(name="p2ms", bufs=3) as p2ms, \
                   tc.tile_pool(name="p2r", bufs=4) as p2r, \
                   tc.tile_pool(name="p2o", bufs=4) as p2o, \
                   tc.tile_pool(name="ps2s", bufs=4, space="PSUM") as ps2s, \
                   tc.tile_pool(name="ps2a", bufs=2, space="PSUM") as ps2a, \
                   tc.tile_pool(name="ps2d", bufs=2, space="PSUM") as ps2d:
                  def load_head(h):
                      kt_h = p2h.tile([128, S], bf16, name="kt_h")
                      qt_h = p2h.tile([128, S], bf16, name="qt_h")
                      nc.scalar.dma_start(out=kt_h, in_=kt_d[h * 128:(h + 1) * 128, :])
                      nc.scalar.dma_start(out=qt_h, in_=qt_d[h * 128:(h + 1) * 128, :])
                      # v^T tiles: [tokens-in-tile, hd] per 128-token tile j
                      vb = p2v.tile([128, NKT, 128], bf16, name="vb")
                      nc.scalar.dma_start(out=vb, in_=vt_d[h])
                      return vb, kt_h, qt_h

                  for h in range(NHL):
                      vb, kt_h, qt_h = load_head(h)
                      v_sb = [vb[:, j, :] for j in range(NKT)]

                      def make_strip(s):
                          c = {
                              "s": s,
                              "act": [j for j in range(NKT) if classes[j][s] != 0],
                              "A": ps2a.tile([128, 512], f32, name="A"),
                              "Dn": ps2d.tile([1, 512], f32, name="Dn"),
                              "qs": qt_h[:, s * 512:(s + 1) * 512],
                              "Es": [], "idx": 0,
                          }
                          return c

                      def emit_scores(c, i):
                          s, j = c["s"], c["act"][i]
                          sps = ps2s.tile([128, 512], f32, name="sps")
                          nc.tensor.matmul(sps, kt_h[:, j * 128:(j + 1) * 128],
                                           c["qs"], start=True, stop=True)
                          E = p2e.tile([128, 512], bf16, name="E")
                          if classes[j][s] == 2:
                              p = j - 4 * s
                              ms = p2ms.tile([128, 512], f32, name="ms")
                              nc.vector.tensor_add(
                                  ms, sps, maskp_sb[:, p * 512:(p + 1) * 512])
                              nc.scalar.activation(E, ms, EXP)
                          else:
                              nc.scalar.activation(E, sps, EXP)
                          c["Es"].append(E)

                      def step(c):
                          """One PV/denominator accumulation step of strip c."""
                          i = c["idx"]
                          if i + depth < len(c["act"]):
                              emit_scores(c, i + depth)
                          j = c["act"][i]
                          first, last = (i == 0), (i == len(c["act"]) - 1)
                          nc.tensor.matmul(c["A"], v_sb[j], c["Es"][i],
                                           start=first, stop=last)
                          nc.tensor.matmul(c["Dn"], ones_k, c["Es"][i],
                                           start=first, stop=last)
                          c["idx"] += 1

                      def normalize(c):
                          rec = p2r.tile([1, 512], f32r, name="rec")
                          nc.vector.reciprocal(rec, c["Dn"][0:1, :])
                          bsb = p2o.tile([128, 512], f32r, name="bsb")
                          nc.gpsimd.partition_broadcast(bsb, rec, 128)
                          s = c["s"]
                          nc.vector.tensor_mul(
                              att_sb[h][:, s * 512:(s + 1) * 512], c["A"], bsb)

                      if not pair_strips:
                          # depth-deep score/exp pipeline: the exp chain for a
                          # diagonal block (mask add + exp) is longer than one
                          # block's PE work, so keep E tiles in flight ahead
                          # of the PV/denominator accumulation.
                          for s in range(TSTRIPS):
                              c = make_strip(s)
                              for d in range(min(depth, len(c["act"]))):
                                  emit_scores(c, d)
                              while c["idx"] < len(c["act"]):
                                  step(c)
                              normalize(c)
                      else:
                          # process strips in pairs: two independent A/Dn
                          # accumulation chains give the PE twice the slack
                          # against the exp-chain latency
                          for sp in range(0, TSTRIPS, 2):
                              cs = [make_strip(sp), make_strip(sp + 1)]
                              for c in cs:
                                  for d in range(min(depth, len(c["act"]))):
                                      emit_scores(c, d)
                              while any(c["idx"] < len(c["act"]) for c in cs):
                                  for c in cs:
                                      if c["idx"] < len(c["act"]):
                                          step(c)
                              for c in cs:
                                  normalize(c)

              # ---------------- Phase 3: output projection ------------------------------
              if 3 in phases:
                with tc.tile_pool(name="p3o", bufs=3) as p3o, \
                   tc.tile_pool(name="ps3", bufs=8, space="PSUM") as ps3:
                  def load_w3(c):
                      wt = p3w.tile([128, NHL, 512], bf16, name="w3")
                      w_ap = wot_d[:, c * 512:(c + 1) * 512].rearrange(
                          "(k p) f -> p k f", p=128)
                      nc.sync.dma_start(out=wt, in_=w_ap)
                      return wt

                  wt_next3 = load_w3(0)
                  am = att_sb
                  for c in range(8):        # dout chunks of 512
                      wt = wt_next3
                      if c + 1 < 8:
                          wt_next3 = load_w3(c + 1)
                      for mg in range(4):   # groups of 4 token tiles
                          st3 = p3o.tile([128, 4, 512], f32, name="st3")
                          for mi in range(4):
                              m = mg * 4 + mi
                              ps = ps3.tile([128, 512], f32, name="ps3")
                              for k in range(NHL):
                                  nc.tensor.matmul(ps, am[k][:, m * 128:(m + 1) * 128],
                                                   wt[:, k, :],
                                                   start=(k == 0), stop=(k == NHL - 1))
                              nc.scalar.activation(st3[:, mi, :], ps, COPY)
                          nc.gpsimd.dma_start(
                              out=out_d[mg * 512:(mg + 1) * 512,
                                        c * 512:(c + 1) * 512].rearrange(
                                  "(m p) f -> p m f", p=128),
                              in_=st3)

    nc.compile()
    return nc


def _host_prep(x, wq, wk, wv, wo, freqs_cos, freqs_sin, mask):
    """Build per-core input maps + mask block classes."""
    import ml_dtypes
    bf16 = ml_dtypes.bfloat16

    x = np.asarray(x, np.float32)
    wq = np.asarray(wq, np.float32)
    wk = np.asarray(wk, np.float32)
    wv = np.asarray(wv, np.float32)
    wo = np.asarray(wo, np.float32)
    mask2 = np.asarray(mask, np.float32).reshape(S, S)

    perm = np.concatenate(
        [hl * 128 + np.concatenate([np.arange(0, 128, 2), np.arange(1, 128, 2)])
         for hl in range(NHL)])
    cosw = np.ascontiguousarray(np.asarray(freqs_cos, np.float32).T)
    sinw = np.ascontiguousarray(np.asarray(freqs_sin, np.float32).T)
    nsinw = np.ascontiguousarray(-sinw)
    maskt = np.ascontiguousarray(mask2.T)

    classes = [[0] * TSTRIPS for _ in range(NKT)]
    for j in range(NKT):
        for s in range(TSTRIPS):
            blk = maskt[j * 128:(j + 1) * 128, s * 512:(s + 1) * 512]
            if (blk <= NEG_THRESH).all():
                classes[j][s] = 0
            elif (blk == 0.0).all():
                classes[j][s] = 1
            else:
                classes[j][s] = 2

    # The partially-masked blocks of a causal mask come in exactly 4 shapes
    # (offset of the 128-row k-block within its 512-col q-strip); keep those
    # resident instead of streaming mask bytes. Verify the assumption holds
    # for the mask we were actually given.
    maskp = np.zeros((128, 4 * 512), np.float32)
    for p in range(4):
        maskp[:, p * 512:(p + 1) * 512] = maskt[p * 128:(p + 1) * 128, 0:512]
    for j in range(NKT):
        for s in range(TSTRIPS):
            if classes[j][s] == 2:
                p = j - 4 * s
                assert 0 <= p < 4, (j, s)
                assert np.array_equal(
                    maskt[j * 128:(j + 1) * 128, s * 512:(s + 1) * 512],
                    maskp[:, p * 512:(p + 1) * 512]), (j, s)

    xts = [np.ascontiguousarray(x[b].T).astype(bf16) for b in range(2)]
    in_maps = []
    for core in range(8):
        b, tp = core // 4, core % 4
        sl = slice(tp * FSH, (tp + 1) * FSH)
        wq_c = wq[sl][perm] * np.float32(1.0 / np.sqrt(HD))
        wk_c = wk[sl][perm]

        def swz(w_c):
            # [D, FSH] (transposed weight) -> [NHL, 128, KT, 128] so each
            # per-head tile loads as one fully-contiguous DMA
            return np.ascontiguousarray(
                w_c.T.reshape(KT, 128, NHL, 128).transpose(2, 1, 0, 3)).astype(bf16)

        in_maps.append({
            "xt": xts[b],
            "wqt": swz(wq_c),
            "wkt": swz(wk_c),
            "wvt": swz(wv[sl]),
            "wot": np.ascontiguousarray(wo[:, sl].T).astype(bf16),
            "cosw": cosw.astype(bf16), "sinw": sinw.astype(bf16),
            "nsinw": nsinw.astype(bf16),
            "maskp": maskp.astype(bf16),
            "ones128": np.ones((128, 128), bf16),
        })
    return in_maps, classes


def kernel(x, wq, wk, wv, wo, freqs_cos, freqs_sin, mask, start_pos=0,
           _trace=False):
    from concourse import bass_utils
    in_maps, classes = _host_prep(x, wq, wk, wv, wo, freqs_cos, freqs_sin, mask)
    key = str(classes)
    if key not in _cache:
        _cache[key] = _build(classes)
    nc = _cache[key]
    res = bass_utils.run_bass_kernel_spmd(nc, in_maps, core_ids=list(range(8)),
                                          trace=_trace)
    out = np.zeros((2, S, D), np.float32)
    for core in range(8):
        out[core // 4] += res.results[core]["out"]
    kernel.last_result = res
    return out


if __name__ == "__main__":
    # compile-only smoke test
    classes = [[2 if j * 128 <= s * 512 + 511 and j * 128 + 127 > s * 512 else
                (1 if j * 128 + 127 <= s * 512 else 0)
                for s in range(TSTRIPS)] for j in range(NKT)]
    import time
    t0 = time.time()
    nc = _build(classes)
    print(f"build+bacc-compile: {time.time()-t0:.1f}s")
    from concourse.timeline_sim import TimelineSim
    est = TimelineSim(nc, trace=False).simulate()
    print(f"TimelineSim: {est:.0f} ns")
    if len(sys.argv) > 1 and sys.argv[1] == "neff":
        import tempfile
        from concourse import bass_utils
        t0 = time.time()
        with tempfile.TemporaryDirectory() as td:
            bass_utils.compile_bass_kernel(nc, td)
            print(f"walrus: {time.time()-t0:.1f}s COMPILED OK")
